# revision 1
# baseline (speedup 1.0000x reference)
"""Trainium2 Bass kernel: 16-head attention with RoPE (dense_transformer).

Sharding: tensor-parallel over heads. 8 cores x 2 heads each.
Each core: Wq/Wk/Wv column slice [1024,128], Wo row slice [128,1024],
full input; computes its heads' attention + partial output projection.
Host sums the 8 partial outputs (row-parallel Wo reduction) and adds bo.

Device layout is "transposed": Q^T/K^T/ctx^T are kept as [dim, seq] with
the head dim on SBUF partitions, so Q^T = Wq^T @ X^T comes straight out
of the PE, scores^T = K^T.T @ Q^T needs no transposes, and the softmax
denominator falls out of an extra ones-column appended to V.
"""

import sys

if "/opt/trn_rl_repo" not in sys.path:
    sys.path.insert(0, "/opt/trn_rl_repo")

import numpy as np
import ml_dtypes

B = 2
S = 2048
NS = B * S  # 4096
D = 1024
H = 16
DK = 64
NCORES = 8
HPC = H // NCORES  # heads per core = 2
DPC = HPC * DK  # model dims per core = 128

_cache = {}


def _build_nc(debug_taps=False):
    import os
    PHASES = int(os.environ.get("K_PHASES", "3"))
    CUTS = set(os.environ.get("K_CUTS", "").split(","))
    import concourse.bass as bass
    import concourse.tile as tile
    import concourse.mybir as mybir
    from concourse import bacc

    fp32 = mybir.dt.float32
    bf16 = mybir.dt.bfloat16
    Exp = mybir.ActivationFunctionType.Exp

    nc = bacc.Bacc("TRN2", debug=False, num_devices=NCORES)
    dbg = {}
    if debug_taps:
        for nm, shape in (
            ("dbg_qrot", [128, NS]),
            ("dbg_krot", [128, NS]),
            ("dbg_v", [128, 32 * 2 * (DK + 1)]),
            ("dbg_ctxT", [128, NS]),
            ("dbg_den", [1, 16 * 512]),
            ("dbg_expS", [128, 16 * 1024]),
            ("dbg_prediv", [128, NS]),
        ):
            dbg[nm] = nc.dram_tensor(nm, shape, bf16 if nm != "dbg_den" else fp32,
                                     kind="ExternalOutput").ap()

    xt = nc.dram_tensor("xt", [D, NS], bf16, kind="ExternalInput").ap()
    wq = nc.dram_tensor("wq", [D, DPC], bf16, kind="ExternalInput").ap()
    wk = nc.dram_tensor("wk", [D, DPC], bf16, kind="ExternalInput").ap()
    wv = nc.dram_tensor("wv", [D, DPC], bf16, kind="ExternalInput").ap()
    wo = nc.dram_tensor("wo", [DPC, D], bf16, kind="ExternalInput").ap()
    bq = nc.dram_tensor("bq", [DPC, 1], fp32, kind="ExternalInput").ap()
    bk = nc.dram_tensor("bk", [DPC, 1], fp32, kind="ExternalInput").ap()
    bv = nc.dram_tensor("bv", [1, DPC], bf16, kind="ExternalInput").ap()
    cos_d = nc.dram_tensor("cos", [128, S], bf16, kind="ExternalInput").ap()
    sin_d = nc.dram_tensor("sin", [128, S], bf16, kind="ExternalInput").ap()
    out_d = nc.dram_tensor("out", [D, NS], bf16, kind="ExternalOutput").ap()

    with tile.TileContext(nc) as tc:
        with (
            tc.tile_pool(name="persist", bufs=1) as persist,
            tc.tile_pool(name="dram", bufs=1, space="DRAM") as dram,
        ):
            qrot = persist.tile([128, NS], bf16, tag="qrot")
            krot = persist.tile([128, NS], bf16, tag="krot")
            # v_sb[:, tt, 65h : 65h+64] = V rows tt*128.. for head h,
            # v_sb[:, tt, 65h+64] = 1.0 (denominator column)
            v_sb = persist.tile([128, 32, 2 * (DK + 1)], bf16, tag="v")
            ctxT = persist.tile([128, NS], bf16, tag="ctxT")
            wo_sb = persist.tile([128, 8, 128], bf16, tag="wo")
            den_sb = persist.tile([1, 16 * 512], fp32, tag="den")
            dn128 = persist.tile([128, 64], fp32, tag="dn128")
            recip128 = persist.tile([128, 64], bf16, tag="recip")
            den_dram = dram.tile([16 * 512], fp32, tag="den_dram")
            rec_dram = dram.tile([16 * 512], bf16, tag="rec_dram")

            nc.sync.dma_start(wo_sb[:], wo.rearrange("p (j m) -> p j m", m=128))

            # ---------------- Phase 1: QKV projections + RoPE ----------------
            with (
                tc.tile_pool(name="qkv_sb", bufs=1) as qkv_sb,
                tc.tile_pool(name="qkv_tmp", bufs=2) as qkv_tmp,
                tc.tile_pool(name="qkv_ps", bufs=4, space="PSUM") as qkv_ps,
                tc.tile_pool(name="qkv_psv", bufs=4, space="PSUM") as qkv_psv,
            ):
                cos_sb = qkv_sb.tile([128, S], bf16, tag="cos")
                sin_sb = qkv_sb.tile([128, S], bf16, tag="sin")
                wq_sb = qkv_sb.tile([128, 8, 128], bf16, tag="wq")
                wk_sb = qkv_sb.tile([128, 8, 128], bf16, tag="wk")
                wv_sb = qkv_sb.tile([128, 8, 128], bf16, tag="wv")
                bq_sb = qkv_sb.tile([128, 1], fp32, tag="bq")
                bk_sb = qkv_sb.tile([128, 1], fp32, tag="bk")
                bvb = qkv_sb.tile([128, 128], bf16, tag="bvb")
                # small operands first so QKV matmuls aren't queued behind xt
                nc.sync.dma_start(wq_sb[:], wq.rearrange("(c p) m -> p c m", p=128))
                nc.sync.dma_start(wk_sb[:], wk.rearrange("(c p) m -> p c m", p=128))
                nc.sync.dma_start(wv_sb[:], wv.rearrange("(c p) m -> p c m", p=128))
                nc.sync.dma_start(bq_sb[:], bq)
                nc.sync.dma_start(bk_sb[:], bk)
                nc.sync.dma_start(bvb[:], bv.to_broadcast((128, 128)))
                nc.gpsimd.dma_start(cos_sb[:], cos_d)
                nc.gpsimd.dma_start(sin_sb[:], sin_d)
                xt_sb = qkv_sb.tile([128, 8, NS], bf16, tag="xt")
                xt_r = xt.rearrange("(c p) s -> p c s", p=128)
                for st in range(8):
                    eng = nc.sync if st % 2 == 0 else nc.gpsimd
                    eng.dma_start(
                        xt_sb[:, :, st * 512 : (st + 1) * 512],
                        xt_r[:, :, st * 512 : (st + 1) * 512],
                    )

                # Q and K: out[d, s] (d on partitions), bias added during
                # the PSUM->SBUF copy, then RoPE on full [128, NS] tiles.
                for w_sb, b_sb, rot in ((wq_sb, bq_sb, qrot), (wk_sb, bk_sb, krot)):
                    plain = qkv_tmp.tile([128, NS], bf16, tag="plain")
                    swap = qkv_tmp.tile([128, NS], bf16, tag="swap")
                    for st in range(8):
                        ps = qkv_ps.tile([128, 512], fp32)
                        for ch in range(8):
                            nc.tensor.matmul(
                                ps[:],
                                w_sb[:, ch, :],
                                xt_sb[:, ch, st * 512 : (st + 1) * 512],
                                start=(ch == 0),
                                stop=(ch == 7),
                            )
                        nc.vector.tensor_scalar_add(
                            plain[:, st * 512 : (st + 1) * 512], ps[:], b_sb[:]
                        )
                    # swap rope halves within each head (cross-partition)
                    for g in (0, 64):
                        nc.sync.dma_start(
                            swap[g : g + 32, :], plain[g + 32 : g + 64, :]
                        )
                        nc.sync.dma_start(
                            swap[g + 32 : g + 64, :], plain[g : g + 32, :]
                        )
                    for b in range(B):
                        sl = slice(b * S, (b + 1) * S)
                        tmp = qkv_tmp.tile([128, S], bf16, tag="ropetmp")
                        nc.vector.tensor_mul(rot[:, sl], plain[:, sl], cos_sb[:])
                        nc.vector.tensor_mul(tmp[:], swap[:, sl], sin_sb[:])
                        nc.vector.tensor_add(rot[:, sl], rot[:, sl], tmp[:])

                # V in normal layout [t, dk], bias via broadcast add on copy
                for tt in range(32):
                    psv = qkv_psv.tile([128, 128], fp32)
                    for ch in range(8):
                        nc.tensor.matmul(
                            psv[:],
                            xt_sb[:, ch, tt * 128 : (tt + 1) * 128],
                            wv_sb[:, ch, :],
                            start=(ch == 0),
                            stop=(ch == 7),
                        )
                    dst = v_sb[:, tt].rearrange("p (h x) -> p h x", h=2)[:, :, 0:DK]
                    nc.vector.tensor_add(dst, psv[:], bvb[:])
                ones_ap = v_sb[:].rearrange("p t (h x) -> p t h x", x=DK + 1)[
                    :, :, :, DK
                ]
                nc.vector.memset(ones_ap, 1.0)


            if debug_taps:
                nc.sync.dma_start(dbg["dbg_qrot"], qrot[:])
                nc.sync.dma_start(dbg["dbg_krot"], krot[:])
                nc.sync.dma_start(dbg["dbg_v"], v_sb[:].rearrange("p a b -> p (a b)"))

            if PHASES >= 2:
                _run_attn = True
            else:
                _run_attn = False
            # ---------------- Phase 2: attention (+ per-batch softmax div) ----
            with (
                tc.tile_pool(name="att_sb", bufs=2) as att_sb,
                tc.tile_pool(name="sc_ps", bufs=2, space="PSUM") as sc_ps,
                tc.tile_pool(name="ctx_ps", bufs=2, space="PSUM") as ctx_ps,
                tc.tile_pool(name="op_ps", bufs=2, space="PSUM") as op_ps,
                tc.tile_pool(name="op_sb", bufs=4) as op_sb,
            ):
                for b in range(B):
                    for h in range(HPC):
                        pi = b * HPC + h
                        hh = h * DK
                        for sh in range(2):  # s-halves of 1024
                            expS = att_sb.tile([128, 16, 1024], bf16, tag="expS")
                            for tt in range(16):
                                ps = sc_ps.tile([128, 1024], fp32)
                                for si in range(2):
                                    s0 = b * S + sh * 1024 + si * 512
                                    nc.tensor.matmul(
                                        ps[:, si * 512 : (si + 1) * 512],
                                        krot[
                                            hh : hh + DK,
                                            b * S + tt * 128 : b * S + (tt + 1) * 128,
                                        ],
                                        qrot[hh : hh + DK, s0 : s0 + 512],
                                        start=True,
                                        stop=True,
                                    )
                                nc.scalar.activation(
                                    expS[:, tt, :], ps[:], Exp, scale=0.125
                                )
                            for sq in range(2):  # 512-wide ctx tiles
                                st_i = sh * 2 + sq
                                pc = ctx_ps.tile([DK + 1, 512], fp32)
                                for tt in range(16):
                                    nc.tensor.matmul(
                                        pc[:],
                                        v_sb[
                                            :,
                                            b * 16 + tt,
                                            h * (DK + 1) : (h + 1) * (DK + 1),
                                        ],
                                        expS[:, tt, sq * 512 : (sq + 1) * 512],
                                        start=(tt == 0),
                                        stop=(tt == 15),
                                    )
                                ds0 = b * S + st_i * 512
                                if h == 0:
                                    nc.vector.tensor_copy(
                                        ctxT[0:DK, ds0 : ds0 + 512], pc[0:DK, :]
                                    )
                                else:
                                    stg = att_sb.tile([DK, 512], bf16, tag="stg")
                                    nc.vector.tensor_copy(stg[:], pc[0:DK, :])
                                    nc.vector.stream_shuffle(
                                        ctxT[DK : 2 * DK, ds0 : ds0 + 512],
                                        stg[:],
                                        mask=list(range(32)),
                                    )
                                nc.vector.tensor_copy(
                                    den_sb[:, (pi * 4 + st_i) * 512 :][:, 0:512],
                                    pc[DK : DK + 1, :],
                                )

                    # per-batch: denominators -> reciprocals -> broadcast mult.
                    # Runs on DMA/DVE while the next batch's attention keeps
                    # PE/ACT busy, so the kernel tail only pays for batch B-1.
                    b0 = b * 2 * S
                    nc.sync.dma_start(
                        den_dram[b0 : b0 + 2 * S].rearrange("(o c) -> o c", o=1),
                        den_sb[0:1, b0 : b0 + 2 * S],
                    )
                    nc.sync.dma_start(
                        dn128[:, b * 32 : (b + 1) * 32],
                        den_dram[b0 : b0 + 2 * S].rearrange("(p c) -> p c", p=128),
                    )
                    with nc.allow_low_precision(
                        reason="bf16 softmax reciprocal is within kernel tolerance"
                    ):
                        nc.vector.reciprocal(
                            recip128[:, b * 32 : (b + 1) * 32],
                            dn128[:, b * 32 : (b + 1) * 32],
                        )
                    nc.sync.dma_start(
                        rec_dram[b0 : b0 + 2 * S].rearrange("(p c) -> p c", p=128),
                        recip128[:, b * 32 : (b + 1) * 32],
                    )
                    R = att_sb.tile([128, S], bf16, tag="R")
                    for h in range(HPC):
                        pi = b * HPC + h
                        nc.sync.dma_start(
                            R[h * DK : (h + 1) * DK, :],
                            rec_dram[pi * S : (pi + 1) * S]
                            .rearrange("(o s) -> o s", o=1)
                            .to_broadcast((DK, S)),
                        )
                    sl = slice(b * S, (b + 1) * S)
                    nc.vector.tensor_mul(ctxT[:, sl], ctxT[:, sl], R[:])

                # ---------------- Phase 3: output projection ----------------
                for st in range(8):
                    for oc in range(8):
                        po = op_ps.tile([128, 512], fp32)
                        nc.tensor.matmul(
                            po[:],
                            wo_sb[:, oc, :],
                            ctxT[:, st * 512 : (st + 1) * 512],
                            start=True,
                            stop=True,
                        )
                        ob = op_sb.tile([128, 512], bf16, tag="ob")
                        if (st * 8 + oc) % 2 == 0:
                            nc.vector.tensor_copy(ob[:], po[:])
                        else:
                            nc.scalar.copy(ob[:], po[:])
                        nc.sync.dma_start(
                            out_d[
                                oc * 128 : (oc + 1) * 128, st * 512 : (st + 1) * 512
                            ],
                            ob[:],
                        )

    nc.compile()
    return nc


def _rope_tables():
    pos = np.arange(S, dtype=np.float64)
    inv_freq = np.exp(np.arange(0, DK, 2, dtype=np.float64) * (-np.log(10000.0) / DK))
    ang = pos[:, None] * inv_freq[None, :]  # [S, 32]
    cos_t = np.empty((128, S), dtype=np.float32)
    sin_t = np.empty((128, S), dtype=np.float32)
    c = np.cos(ang).astype(np.float32).T  # [32, S]
    s = np.sin(ang).astype(np.float32).T
    for blk in range(4):
        cos_t[blk * 32 : (blk + 1) * 32] = c
        sign = -1.0 if blk % 2 == 0 else 1.0
        sin_t[blk * 32 : (blk + 1) * 32] = sign * s
    return cos_t, sin_t


def _prep_inputs(inputs, Wq, bq, Wk, bk, Wv, bv, Wo):
    bf = ml_dtypes.bfloat16
    x2 = np.asarray(inputs, dtype=np.float32).reshape(NS, D)
    xt = np.ascontiguousarray(x2.T).astype(bf)
    cos_t, sin_t = _rope_tables()
    cos_b = cos_t.astype(bf)
    sin_b = sin_t.astype(bf)
    in_maps = []
    for c in range(NCORES):
        sl = slice(c * DPC, (c + 1) * DPC)
        in_maps.append(
            {
                "xt": xt,
                "wq": np.ascontiguousarray(Wq[:, sl]).astype(bf),
                "wk": np.ascontiguousarray(Wk[:, sl]).astype(bf),
                "wv": np.ascontiguousarray(Wv[:, sl]).astype(bf),
                "wo": np.ascontiguousarray(Wo[sl, :]).astype(bf),
                "bq": np.ascontiguousarray(bq[sl]).reshape(DPC, 1).astype(np.float32),
                "bk": np.ascontiguousarray(bk[sl]).reshape(DPC, 1).astype(np.float32),
                "bv": np.ascontiguousarray(bv[sl]).reshape(1, DPC).astype(bf),
                "cos": cos_b,
                "sin": sin_b,
            }
        )
    return in_maps


def _get_nc():
    if "nc" not in _cache:
        _cache["nc"] = _build_nc()
    return _cache["nc"]


def run(inputs_dict, trace=False):
    """Build (cached), run on 8 cores, assemble full output. Returns
    (output fp32 [B,S,D], BassKernelResults)."""
    from concourse.bass_utils import run_bass_kernel_spmd

    nc = _get_nc()
    in_maps = _prep_inputs(
        inputs_dict["inputs"],
        inputs_dict["Wq"],
        inputs_dict["bq"],
        inputs_dict["Wk"],
        inputs_dict["bk"],
        inputs_dict["Wv"],
        inputs_dict["bv"],
        inputs_dict["Wo"],
    )
    res = run_bass_kernel_spmd(
        nc, in_maps, core_ids=list(range(NCORES)), trace=trace
    )
    acc = np.zeros((D, NS), dtype=np.float32)
    for r in res.results:
        acc += r["out"].astype(np.float32)
    out = acc.T.reshape(B, S, D) + np.asarray(inputs_dict["bo"], dtype=np.float32)
    return out.astype(np.float32), res


def kernel(**inputs):
    out, _ = run(inputs, trace=False)
    return out

